# revision 10
# baseline (speedup 1.0000x reference)
"""Complex per-mode matmul: out[b,o,x,y] = sum_i in[b,i,x,y] * w[i,o,x,y] (complex).

Shapes (hardcoded): input [32,128,64,65,2] f32, weight [128,128,64,65,2] f32,
output [32,128,64,65,2] f32, where the trailing 2 is (real, imag).

Strategy:
  - Shard the 64 x-modes across 8 cores (8 per core). Contraction is over
    in_channels for each (x,y) independently, so this needs zero replication
    and no collectives: per-core I/O is 1/8 of everything.
  - Per mode (x,y): psum[o, b*2+c] accumulates two matmuls
        MM1: lhsT=Wr[i,o] (128 cols), rhs cols (Xr[i,b], Xi[i,b]) pairs  -> (Wr.Xr, Wr.Xi)
        MM2: lhsT=Wi[i,o],           rhs cols (-Xi[i,b], Xr[i,b]) pairs  -> (-Wi.Xi, Wi.Xr)
    giving out_r = Wr.Xr - Wi.Xi at even cols, out_i = Wr.Xi + Wi.Xr at odd.
    The -Xi copy is precomputed on host so PSUM accumulation does the subtract.
  - Inputs are cast to fp16 on host (PSUM accumulates in fp32); output fp32.
  - Host pre-transposes both operands so every DMA moves large contiguous
    per-partition lines; w and x are concatenated into one DRAM tensor so a
    single DMA per x-slice brings in all inputs:
      cin layout [x][i (part)][w: c(2), y(65), o(128) | x: b(32), y(65), c3(3)]
      out layout [o (part)][x][b(32), y(65), c(2)]    fp32
  - This walrus build fits only ONE sync wait per hardware instruction; a
    post-pass splits any extra waits into standalone EventSemaphore
    instructions on the same engine queue (the wait-carrier bacc uses).
"""

import numpy as np

B, CIN, COUT, M1, M2 = 32, 128, 128, 64, 65
NCORES = 8
XPC = M1 // NCORES  # x-slices per core
MPG = 8  # modes per PSUM bank (8 * 64 cols = 512 = one bank)


def _split_excess_waits(nc, mybir):
    """Walrus codegen fits one sync wait per instruction; move extras onto
    EventSemaphore instructions inserted just before, on the same engine."""
    n = 0
    for fn in nc.m.functions:
        for blk in fn.blocks:
            out = []
            for inst in blk.instructions:
                si = inst.sync_info
                if si is not None and si.on_wait and len(si.on_wait) > 1:
                    waits = list(si.on_wait)
                    for w in waits[:-1]:
                        ev = mybir.InstEventSemaphore(
                            name=f"evsplit_{n}",
                            engine=inst.engine,
                            ins=[],
                            outs=[],
                            sync_info=mybir.SyncInfo(on_wait=[w], on_update=[]),
                            bass_nofuse=True,
                        )
                        n += 1
                        out.append(ev)
                    si.on_wait = [waits[-1]]
                out.append(inst)
            blk.instructions = out


def build_nc(xpc=XPC, b=B, yc=M2, cout=COUT):
    import concourse.bass as bass
    import concourse.mybir as mybir
    from concourse.tile import TileContext

    dt = mybir.dt.float16
    f32 = mybir.dt.float32
    WW = 2 * yc * cout  # weight cols per slice
    XW = b * yc * 3  # input cols per slice
    nc = bass.Bass()
    cin = nc.dram_tensor("cin", [xpc, CIN, WW + XW], dt, kind="ExternalInput")
    out = nc.dram_tensor("out", [cout, xpc, b * yc * 2], f32, kind="ExternalOutput")

    groups = [(g0, min(MPG, yc - g0)) for g0 in range(0, yc, MPG)]

    with TileContext(nc) as tc:
        with (
            tc.tile_pool(name="cpool", bufs=2) as cpool,
            tc.tile_pool(name="opool", bufs=2) as opool,
            tc.tile_pool(name="ppool", bufs=4, space="PSUM") as ppool,
        ):
            for x in range(xpc):
                ctile = cpool.tile([CIN, WW + XW], dt, name="ctile")
                nc.sync.dma_start(out=ctile, in_=cin[x])
                wv = ctile[:, :WW].rearrange("p (c y o) -> p c y o", c=2, y=yc)
                xv = ctile[:, WW:].rearrange("p (b y c) -> p b y c", b=b, y=yc)
                otile = opool.tile([cout, b * yc * 2], f32, name="otile")
                # view with y outermost so psum groups map to strided y-slices
                ov = otile.rearrange("p (b y c) -> p y b c", b=b, y=yc)
                for y0, gs in groups:
                    ptile = ppool.tile([cout, 512], f32, name="ptile")
                    for m in range(gs):
                        y = y0 + m
                        ps = ptile[:, m * 2 * b : (m + 1) * 2 * b]
                        nc.tensor.matmul(
                            ps, wv[:, 0, y, :], xv[:, :, y, 1:3],
                            start=True, stop=False,
                        )
                        nc.tensor.matmul(
                            ps, wv[:, 1, y, :], xv[:, :, y, 0:2],
                            start=False, stop=True,
                        )
                    pv = ptile[:, : gs * 2 * b].rearrange(
                        "p (m b c) -> p m b c", m=gs, b=b
                    )
                    nc.vector.tensor_copy(out=ov[:, y0 : y0 + gs], in_=pv)
                nc.sync.dma_start(out=out[:, x, :], in_=otile)

    _split_excess_waits(nc, mybir)
    return nc


def prep_inputs(input, weight):
    """Host-side re-layout + fp16 cast. Returns combined [64, 128, 16640+6240]."""
    # weight [i,o,x,y,c] -> [x,i,c,y,o]
    w16 = weight.transpose(2, 0, 4, 3, 1).astype(np.float16)
    w16 = w16.reshape(M1, CIN, 2 * M2 * COUT)
    xr = input[..., 0]
    xi = input[..., 1]
    st = np.stack([-xi, xr, xi], axis=-1)  # [b,i,x,y,3]
    x16 = st.transpose(2, 1, 0, 3, 4).astype(np.float16)  # [x,i,b,y,3]
    x16 = x16.reshape(M1, CIN, B * M2 * 3)
    return np.concatenate([w16, x16], axis=2)


def gather_output(per_core):
    """per_core: list of 8 arrays [cout, xpc, b*yc*2] fp32 -> [B, COUT, M1, M2, 2]."""
    out = np.empty((B, COUT, M1, M2, 2), np.float32)
    for k, arr in enumerate(per_core):
        a = arr.reshape(COUT, XPC, B, M2, 2)  # [o, x, b, y, c]
        out[:, :, k * XPC : (k + 1) * XPC] = a.transpose(2, 0, 1, 3, 4)
    return out


_NC = None
TRACE = False  # test harness can set True to collect a HW profile
LAST_RESULTS = None


def kernel(input, weight):
    global _NC, LAST_RESULTS
    from concourse.bass_utils import run_bass_kernel_spmd

    if _NC is None:
        _NC = build_nc()
    c16 = prep_inputs(np.asarray(input), np.asarray(weight))
    in_maps = [
        {"cin": np.ascontiguousarray(c16[k * XPC : (k + 1) * XPC])}
        for k in range(NCORES)
    ]
    res = run_bass_kernel_spmd(_NC, in_maps, core_ids=list(range(NCORES)), trace=TRACE)
    LAST_RESULTS = res
    return gather_output([r["out"] for r in res.results])
